# Initial kernel scaffold
#
"""Distributed GAT (AnomalyDAE encoder) kernel for 8 TRN2 NeuronCores.

Reference computation:
    h = leaky_relu(x @ W_dense.T + b_dense, 0.01)          # [N, 128]
    g = h @ W_gat.T                                        # [N, 64]
    a_src = g @ att_src ; a_dst = g @ att_dst              # [N]
    with self-loops appended, per edge (s -> d):
        e = leaky_relu(a_src[s] + a_dst[d], 0.2)
        alpha = segment_softmax(e, by d)
    out[d] = sum_e alpha_e * g[s_e] + b_gat                # [N, 64]

Sharding: nodes split contiguously across 8 cores (6250 each); edges
partitioned by destination core. Per-core nodes are degree-sorted so
128-node tiles have near-uniform degree.

Key idea vs the 2-half baseline: a SINGLE table (50176 rows of 256 B:
[g bf16 x64 | u=e^{a_src} | v=e^{0.2 a_src} | pad]) gathered with
per-call base offsets. Within each dst its slots are source-sorted, so
the k-th slot across a tile's 128 dsts concentrates in a gid quantile
band; each 8-strip gather call gets its own 32768-row window (int16
index range), eliminating the source-half split (-29% gathered bytes).
The few out-of-window "straggler" edges go to per-tile overflow strips.
Edge weight on-chip: w = max(u*p_d, v*q_d) with p=e^{a_dst}, q=e^{0.2
a_dst} per-partition scalars (exp moved to the node phase; identity
e^{lrelu(x)} = max(e^x, e^{0.2x}), softmax computed without max-shift).
Pad slots point at a pad row with u=v=0 so their weight is exactly 0.
"""

import numpy as np
import ml_dtypes

bf16 = ml_dtypes.bfloat16

R = 8            # cores
P = 128          # partitions / tile size
W_ROW = 128      # table row width in bf16 elems (256 B)
ASRC_F32 = 32    # f32 column of a_src within a row (byte offset 128)
WIN = 32768      # int16 index window (rows per gather call)
STRIP = 8        # k-slots per gather call (8*128 = 1024 descriptors)


class Cfg:
    def __init__(self, N, E, IN=512, EMB=128, OUT=64):
        assert N % R == 0
        self.N, self.E, self.IN, self.EMB, self.OUT = N, E, IN, EMB, OUT
        self.NL = N // R
        self.NL_pad = ((self.NL + 2 + P - 1) // P) * P
        self.TILES = self.NL_pad // P
        self.NTAB = self.NL_pad * R
        self.NCH = 7                         # all-gather chunks
        assert self.TILES % self.NCH == 0
        self.TPC = self.TILES // self.NCH    # tiles per chunk
        self.CH = self.NL_pad // self.NCH    # rows per chunk per core
        npad = self.NL_pad - self.NL
        base_pads = npad // self.NCH
        self.pads = [base_pads + (1 if c < npad % self.NCH else 0)
                     for c in range(self.NCH)]
        pos = np.arange(self.NL_pad)
        inchunk = pos % self.CH
        self.is_pad = np.zeros(self.NL_pad, bool)
        for c in range(self.NCH):
            self.is_pad[(pos // self.CH == c)
                        & (inchunk >= self.CH - self.pads[c])] = True
        self.positions_real = pos[~self.is_pad]
        assert len(self.positions_real) == self.NL


CFG_REAL = Cfg(N=50000, E=1600000)


# --------------------------------------------------------------------------
# host-side preprocessing
# --------------------------------------------------------------------------

def _wrap_idx(lin):
    """dma_gather index layout: linear i -> [i % 16, i // 16], replicated
    across the 8 Q7 core groups -> [128, len/16] int16."""
    assert len(lin) % 16 == 0
    w = lin.reshape(-1, 16).T.astype(np.int16)
    return np.tile(w, (8, 1))


def _pad_row_in(base, cfg):
    """A table row with a_src sentinel inside [base, base+WIN): the last
    row of every (chunk, core) subblock is a pad row."""
    s = base + ((cfg.CH - 1 - base) % cfg.CH)
    assert base <= s < base + WIN and s < cfg.NTAB
    return s


def _build_layout(cfg, src, dst, deg):
    """Slot grids, call windows and per-core index blocks (graph metadata
    is core-uniform; index contents are per-core)."""
    N, NL, NL_pad, TILES = cfg.N, cfg.NL, cfg.NL_pad, cfg.TILES

    CH, NCH = cfg.CH, cfg.NCH
    pos_of = np.empty(N, dtype=np.int64)
    orders = []
    for r in range(R):
        dloc = deg[r * NL:(r + 1) * NL]
        order = np.argsort(-dloc, kind="stable")
        orders.append(order)
        pos_of[r * NL + order] = cfg.positions_real
    core_of = np.arange(N) // NL
    # chunk-major table: gid = chunk*R*CH + core*CH + pos%CH
    gid_of = (pos_of // CH) * R * CH + core_of * CH + (pos_of % CH)

    grids = []
    for r in range(R):
        m = (dst >= r * NL) & (dst < (r + 1) * NL)
        s_r = gid_of[src[m]]
        dpos = pos_of[dst[m]]
        order = np.lexsort((s_r, dpos))
        s_s, d_s = s_r[order], dpos[order]
        starts = np.zeros(NL_pad + 1, np.int64)
        np.add.at(starts, d_s + 1, 1)
        starts = np.cumsum(starts)
        k_of = np.arange(len(s_s)) - starts[d_s]
        grids.append((s_s, d_s, k_of))

    degs_sorted = [np.sort(deg[r * NL:(r + 1) * NL])[::-1] for r in range(R)]
    tile_of_rank = cfg.positions_real // P        # rank i -> tile
    D = np.ones(TILES, np.int64)
    for t in range(TILES):
        rk = np.nonzero(tile_of_rank == t)[0]
        if rk.size:
            D[t] = max(max(int(ds[rk].max()) for ds in degs_sorted), 1)

    tile_meta = []      # per tile: (D_eff, [(k0, nk, base), ...])
    idx_blocks = [[] for _ in range(R)]   # per core: list of wrapped blocks

    for t in range(TILES):
        Dt = int(D[t])
        G = np.full((R, P, Dt), -1, np.int64)
        for r in range(R):
            s_s, d_s, k_of = grids[r]
            mm = (d_s >= t * P) & (d_s < (t + 1) * P)
            G[r, d_s[mm] - t * P, k_of[mm]] = s_s[mm]

        calls = []
        stragglers = []   # (r, d, gid)

        def emit(k0, nk):
            blk = G[:, :, k0:k0 + nk]
            v = blk[blk >= 0]
            if v.size == 0:
                calls.append((k0, nk, 0))
                return
            vs = np.sort(v)
            if vs[-1] - vs[0] < WIN:
                calls.append((k0, nk, int(vs[0])))
                return
            if nk > 1:
                h = nk // 2
                emit(k0, h)
                emit(k0 + h, nk - h)
                return
            # single strip still over-span: max-coverage window
            hi_idx = np.searchsorted(vs, vs + WIN, side="left")
            cover = hi_idx - np.arange(len(vs))
            i0 = int(np.argmax(cover))
            base = int(vs[i0])
            bad = (blk >= 0) & ((blk < base) | (blk >= base + WIN))
            for r, dd, kk in zip(*np.nonzero(bad)):
                stragglers.append((int(r), int(dd), int(blk[r, dd, kk])))
                G[r, dd, k0 + kk] = -1
            calls.append((k0, nk, base))

        for k0 in range(0, Dt, STRIP):
            emit(k0, min(STRIP, Dt - k0))

        # re-home stragglers: free (sentinel) slot in a window-matching
        # call, else swap with a same-dst edge that fits both windows
        leftover = []
        for r, dd, gid in stragglers:
            placed = False
            for (k0, nk, base) in calls:
                if base <= gid < base + WIN:
                    free = np.nonzero(G[r, dd, k0:k0 + nk] < 0)[0]
                    if free.size:
                        G[r, dd, k0 + int(free[0])] = gid
                        placed = True
                        break
            if not placed:
                for (k0, nk, base) in calls:
                    if placed or not (base <= gid < base + WIN):
                        continue
                    for kk in range(nk):
                        other = int(G[r, dd, k0 + kk])
                        if other < 0:
                            continue
                        # move `other` elsewhere so `gid` can take its slot
                        for (k2, nk2, base2) in calls:
                            if (k2, nk2, base2) == (k0, nk, base):
                                continue
                            if base2 <= other < base2 + WIN:
                                free2 = np.nonzero(
                                    G[r, dd, k2:k2 + nk2] < 0)[0]
                                if free2.size:
                                    G[r, dd, k2 + int(free2[0])] = other
                                    G[r, dd, k0 + kk] = gid
                                    placed = True
                                    break
                        if placed:
                            break
            if not placed:
                leftover.append((r, dd, gid))
        # overflow strips (shared windows across cores)
        ov = []   # list of dicts: {base, slots: {(r, d): gid}}
        leftover.sort(key=lambda x: x[2])
        for r, dd, gid in leftover:
            for s in ov:
                if s["base"] <= gid < s["base"] + WIN and (r, dd) not in s["slots"]:
                    s["slots"][(r, dd)] = gid
                    break
            else:
                ov.append({"base": gid, "slots": {(r, dd): gid}})
        Dov = len(ov)
        De = Dt + Dov
        Gf = np.full((R, P, De), -1, np.int64)
        Gf[:, :, :Dt] = G
        for j, s in enumerate(ov):
            for (r, dd), gid in s["slots"].items():
                Gf[r, dd, Dt + j] = gid
            calls.append((Dt + j, 1, int(s["base"])))

        tile_meta.append((De, calls))

        for r in range(R):
            for (k0, nk, base) in calls:
                pad = _pad_row_in(base, cfg)
                lin = Gf[r, :, k0:k0 + nk].T.reshape(-1).copy()  # i = kk*128+p
                lin[lin < 0] = pad
                lin -= base
                assert lin.min() >= 0 and lin.max() < WIN
                idx_blocks[r].append(_wrap_idx(lin))

    offs = [np.ascontiguousarray(np.concatenate(b, axis=1)) for b in idx_blocks]
    return orders, gid_of, tile_meta, offs


def _prepare(cfg, x, edge_index, W_dense, b_dense, W_gat, att_src, att_dst,
             b_gat):
    N, NL, NL_pad, TILES = cfg.N, cfg.NL, cfg.NL_pad, cfg.TILES
    src = edge_index[0].astype(np.int64)
    dst = edge_index[1].astype(np.int64)
    loops = np.arange(N, dtype=np.int64)
    src = np.concatenate([src, loops])
    dst = np.concatenate([dst, loops])
    deg = np.bincount(dst, minlength=N)

    orders, gid_of, tile_meta, offs = _build_layout(cfg, src, dst, deg)

    wdT = np.ascontiguousarray(W_dense.T)            # [IN, EMB]
    wdT_packed = np.concatenate(
        [wdT[k * P:(k + 1) * P, :] for k in range(cfg.IN // P)], axis=1)
    att = np.stack([att_src, att_dst], axis=1)       # [OUT, 2]
    KC = cfg.IN // P

    in_maps = []
    for r in range(R):
        xp = np.zeros((NL_pad, cfg.IN), dtype=np.float32)
        xp[cfg.positions_real] = x[r * NL + orders[r]]
        xT = np.empty((P, TILES * KC * P), dtype=bf16)
        for t in range(TILES):
            blk = xp[t * P:(t + 1) * P, :].T.astype(bf16)   # [IN, P]
            xT[:, (t * KC) * P:(t + 1) * KC * P] = \
                blk.reshape(KC, P, P).transpose(1, 0, 2).reshape(P, KC * P)
        in_maps.append({
            "xT": xT,
            "wdT": wdT_packed.astype(bf16),
            "bd": b_dense.reshape(cfg.EMB, 1).astype(np.float32),
            "wgT": np.ascontiguousarray(W_gat.T).astype(bf16),
            "att": att.astype(bf16),
            "bgat": b_gat.reshape(cfg.OUT, 1).astype(np.float32),
            "offs": offs[r],
        })
    return in_maps, orders, tile_meta


def _assemble(cfg, results, orders):
    out = np.empty((cfg.N, cfg.OUT), dtype=np.float32)
    for r in range(R):
        o = results[r]["out"][cfg.positions_real]
        out[r * cfg.NL + orders[r]] = o
    return out


# --------------------------------------------------------------------------
# device graph
# --------------------------------------------------------------------------

def _build_graph(cfg, tile_meta):
    import concourse.bass as bass
    import concourse.bacc as bacc
    import concourse.mybir as mybir
    import concourse.tile as tile
    from concourse.masks import make_identity

    IN, EMB, OUT = cfg.IN, cfg.EMB, cfg.OUT
    KC = IN // P
    TILES, NL_pad, NTAB, NL = cfg.TILES, cfg.NL_pad, cfg.NTAB, cfg.NL
    TOTD = sum(m[0] for m in tile_meta)
    fp32 = mybir.dt.float32
    b16 = mybir.dt.bfloat16
    i16 = mybir.dt.int16

    nc = bacc.Bacc(None, target_bir_lowering=False, debug=False, num_devices=R,
                   num_swdge_queues=4)

    xT = nc.dram_tensor("xT", [P, TILES * KC * P], b16, kind="ExternalInput")
    wdT = nc.dram_tensor("wdT", [P, KC * EMB], b16, kind="ExternalInput")
    bd = nc.dram_tensor("bd", [EMB, 1], fp32, kind="ExternalInput")
    wgT = nc.dram_tensor("wgT", [EMB, OUT], b16, kind="ExternalInput")
    att = nc.dram_tensor("att", [OUT, 2], b16, kind="ExternalInput")
    bgat = nc.dram_tensor("bgat", [OUT, 1], fp32, kind="ExternalInput")
    offs_ext = nc.dram_tensor("offs", [P, 8 * TOTD], i16, kind="ExternalInput")
    out = nc.dram_tensor("out", [NL_pad, OUT], fp32, kind="ExternalOutput")

    with tile.TileContext(nc) as tc:
        with (
            tc.tile_pool(name="dram", bufs=1, space="DRAM") as dram,
            tc.tile_pool(name="const", bufs=1) as cst,
        ):
            shard = dram.tile([NL_pad, W_ROW], b16)
            full = dram.tile([NTAB, W_ROW], b16)

            identb = cst.tile([P, P], b16)
            make_identity(nc, identb[:])
            identf = cst.tile([P, P], fp32)
            make_identity(nc, identf[:])

            wdTs = cst.tile([P, KC * EMB], b16)
            nc.sync.dma_start(out=wdTs[:], in_=wdT[:, :])
            bds = cst.tile([EMB, 1], fp32)
            nc.sync.dma_start(out=bds[:], in_=bd[:, :])
            wgTs = cst.tile([EMB, OUT], b16)
            nc.sync.dma_start(out=wgTs[:], in_=wgT[:, :])
            atts = cst.tile([OUT, 2], b16)
            nc.sync.dma_start(out=atts[:], in_=att[:, :])
            attmat = cst.tile([P, 2 * OUT], b16)
            bgs = cst.tile([OUT, 1], fp32)
            nc.sync.dma_start(out=bgs[:], in_=bgat[:, :])
            adst_all = cst.tile([P, TILES], fp32)
            zuv = cst.tile([max(cfg.pads), 1], fp32)
            nc.vector.memset(zuv[:], -80.0)
            adst02_all = cst.tile([P, TILES], fp32)

            # ---------------- node phase ----------------
            with (
                tc.tile_pool(name="npsum_h", bufs=2, space="PSUM") as ps_h,
                tc.tile_pool(name="npsum_m", bufs=1, space="PSUM") as ps_m,
                tc.tile_pool(name="nsb", bufs=3) as nsb,
            ):
                bgp = ps_m.tile([P, OUT], fp32, tag="misc")
                nc.tensor.transpose(out=bgp[:], in_=bgs[:].to_broadcast([OUT, P]),
                                    identity=identf[:OUT, :OUT])
                bgmat = cst.tile([P, OUT], fp32)
                nc.vector.tensor_copy(bgmat[:], bgp[:])
                # att rows replicated across partitions: [att_src | att_dst]
                amp = ps_m.tile([P, 2 * OUT], b16, tag="misc")
                nc.tensor.transpose(
                    out=amp[:, 0:OUT], in_=atts[:, 0:1].to_broadcast([OUT, P]),
                    identity=identb[:OUT, :OUT])
                nc.tensor.transpose(
                    out=amp[:, OUT:2 * OUT],
                    in_=atts[:, 1:2].to_broadcast([OUT, P]),
                    identity=identb[:OUT, :OUT])
                nc.vector.tensor_copy(attmat[:], amp[:])

                for t in range(TILES):
                    xTs = nsb.tile([P, KC * P], b16, tag="xTs")
                    nc.sync.dma_start(
                        out=xTs[:], in_=xT[:, t * KC * P:(t + 1) * KC * P])
                    hTp = ps_h.tile([EMB, P], fp32, tag="hT")
                    for k in range(KC):
                        nc.tensor.matmul(out=hTp[:],
                                         lhsT=wdTs[:, k * EMB:(k + 1) * EMB],
                                         rhs=xTs[:, k * P:(k + 1) * P],
                                         start=(k == 0), stop=(k == KC - 1))
                    u = nsb.tile([EMB, P], fp32, tag="u")
                    nc.scalar.activation(u[:], hTp[:],
                                         mybir.ActivationFunctionType.Identity,
                                         bias=bds[:, :1])
                    hT = nsb.tile([EMB, P], b16, tag="hT_sb")
                    nc.vector.scalar_tensor_tensor(
                        out=hT[:], in0=u[:], scalar=0.01, in1=u[:],
                        op0=mybir.AluOpType.mult, op1=mybir.AluOpType.max)
                    gTp = ps_m.tile([OUT, P], fp32, tag="misc")
                    nc.tensor.matmul(out=gTp[:], lhsT=wgTs[:], rhs=hT[:],
                                     start=True, stop=True)
                    stg = nsb.tile([OUT, P], b16, tag="stg")
                    nc.vector.tensor_copy(stg[:], gTp[:])
                    # transpose gT -> table g block
                    ttp = ps_m.tile([P, OUT], b16, tag="ttp")
                    nc.tensor.transpose(out=ttp[:], in_=stg[:],
                                        identity=identb[:OUT, :OUT])
                    tabs = nsb.tile([P, W_ROW], b16, tag="tabs")
                    nc.scalar.activation(tabs[:, 0:OUT], ttp[:],
                                         mybir.ActivationFunctionType.Copy)
                    # a_src/a_dst as per-partition row-dots with att columns
                    gw = nsb.tile([P, 2 * OUT], fp32, tag="gw")
                    nc.vector.tensor_tensor(
                        out=gw[:, 0:OUT], in0=tabs[:, 0:OUT],
                        in1=attmat[:, 0:OUT], op=mybir.AluOpType.mult)
                    nc.vector.tensor_tensor(
                        out=gw[:, OUT:2 * OUT], in0=tabs[:, 0:OUT],
                        in1=attmat[:, OUT:2 * OUT], op=mybir.AluOpType.mult)
                    nc.vector.tensor_reduce(
                        out=tabs[:].bitcast(fp32)[:, ASRC_F32:ASRC_F32 + 1],
                        in_=gw[:, 0:OUT], op=mybir.AluOpType.add,
                        axis=mybir.AxisListType.X)
                    nc.vector.tensor_reduce(
                        out=adst_all[:, t:t + 1],
                        in_=gw[:, OUT:2 * OUT], op=mybir.AluOpType.add,
                        axis=mybir.AxisListType.X)
                    nc.sync.dma_start(
                        out=shard[t * P:(t + 1) * P, :], in_=tabs[:])

                    # chunk boundary: pad sentinels; all-gather in 2 groups
                    if (t + 1) % cfg.TPC == 0:
                        c = t // cfg.TPC
                        pc = cfg.pads[c]
                        lo = (c + 1) * cfg.CH - pc
                        nc.sync.dma_start(
                            out=shard[lo:lo + pc, :].bitcast(fp32)
                                [:, ASRC_F32:ASRC_F32 + 1],
                            in_=zuv[:pc, :])
                        nc.gpsimd.collective_compute(
                            "AllGather", mybir.AluOpType.bypass,
                            replica_groups=[list(range(R))],
                            ins=[shard[c * cfg.CH:(c + 1) * cfg.CH, :].opt()],
                            outs=[full[c * R * cfg.CH:
                                       (c + 1) * R * cfg.CH, :].opt()],
                        )

                nc.vector.tensor_scalar(
                    out=adst02_all[:], in0=adst_all[:], scalar1=0.2,
                    scalar2=None, op0=mybir.AluOpType.mult)

            # ---------------- edge phase ----------------
            qi = 0
            allidx = cst.tile([P, 8 * TOTD], i16)
            nc.sync.dma_start(out=allidx[:], in_=offs_ext[:, :])
            with tc.tile_pool(name="esb", bufs=3) as esb:
                cum = 0
                for t in range(TILES):
                    De, calls = tile_meta[t]
                    Dq = De
                    rows = esb.tile([P, Dq * W_ROW], b16, tag="rows")
                    for (k0, nk, base) in calls:
                        hi = min(base + WIN, NTAB)
                        nc.gpsimd.dma_gather(
                            out_ap=rows[:, k0 * W_ROW:(k0 + nk) * W_ROW]
                                .rearrange("p (j e) -> p j e", e=W_ROW),
                            in_ap=full[base:hi, :],
                            idxs_ap=allidx[:, 8 * (cum + k0):
                                           8 * (cum + k0 + nk)],
                            num_idxs=nk * P, num_idxs_reg=nk * P,
                            elem_size=W_ROW,
                            single_packet=False,
                            queue_num=qi % 4,
                        )
                        qi += 1
                    rowsv = rows[:].rearrange("p (d e) -> p d e", e=W_ROW)
                    asrc = rows[:].bitcast(fp32).rearrange(
                        "p (d e) -> p d e", e=W_ROW // 2)[:, :, ASRC_F32]
                    # w = e^{lrelu(a_src+a_dst, 0.2)} = max(e^s, e^{0.2 s})
                    t1 = esb.tile([P, Dq], fp32, tag="t1")
                    nc.scalar.activation(t1[:], asrc,
                                         mybir.ActivationFunctionType.Exp,
                                         bias=adst_all[:, t:t + 1], scale=1.0)
                    t2 = esb.tile([P, Dq], fp32, tag="t2")
                    nc.scalar.activation(t2[:], asrc,
                                         mybir.ActivationFunctionType.Exp,
                                         bias=adst02_all[:, t:t + 1], scale=0.2)
                    w = esb.tile([P, Dq], fp32, tag="w")
                    nc.vector.tensor_tensor(out=w[:], in0=t1[:], in1=t2[:],
                                            op=mybir.AluOpType.max)
                    denom = esb.tile([P, 1], fp32, tag="denom")
                    nc.vector.tensor_reduce(out=denom[:], in_=w[:],
                                            op=mybir.AluOpType.add,
                                            axis=mybir.AxisListType.X)
                    rden = esb.tile([P, 1], fp32, tag="rden")
                    nc.vector.reciprocal(rden[:], denom[:])
                    gsc = esb.tile([P, Dq * OUT], b16, tag="gsc")
                    nc.vector.tensor_tensor(
                        out=gsc[:].rearrange("p (d c) -> p d c", c=OUT),
                        in0=rowsv[:, :, 0:OUT],
                        in1=w[:].to_broadcast([P, Dq, OUT]),
                        op=mybir.AluOpType.mult)
                    onum = esb.tile([P, OUT], fp32, tag="onum")
                    nc.vector.tensor_reduce(
                        out=onum[:],
                        in_=gsc[:].rearrange("p (d c) -> p c d", c=OUT),
                        op=mybir.AluOpType.add, axis=mybir.AxisListType.X)
                    outf = esb.tile([P, OUT], fp32, tag="outf")
                    nc.vector.scalar_tensor_tensor(
                        out=outf[:], in0=onum[:], scalar=rden[:, :1],
                        in1=bgmat[:],
                        op0=mybir.AluOpType.mult, op1=mybir.AluOpType.add)
                    nc.sync.dma_start(out=out[t * P:(t + 1) * P, :], in_=outf[:])
                    cum += De
    nc.finalize()
    return nc


# --------------------------------------------------------------------------
# entry points
# --------------------------------------------------------------------------

def run(inputs, cfg=CFG_REAL, trace=False):
    from concourse.bass_utils import run_bass_kernel_spmd
    in_maps, orders, tile_meta = _prepare(cfg, **inputs)
    nc = _build_graph(cfg, tile_meta)
    res = run_bass_kernel_spmd(nc, in_maps, core_ids=list(range(R)),
                               trace=trace)
    out = _assemble(cfg, res.results, orders)
    return out, res


def kernel(**inputs):
    inputs = {k: np.asarray(v) for k, v in inputs.items()}
    out, _ = run(inputs, CFG_REAL, trace=False)
    return out



# revision 1
# speedup vs baseline: 3.5719x; 3.5719x over previous
"""Distributed GAT (AnomalyDAE encoder) kernel for 8 TRN2 NeuronCores.

Reference computation:
    h = leaky_relu(x @ W_dense.T + b_dense, 0.01)          # [N, 128]
    g = h @ W_gat.T                                        # [N, 64]
    a_src = g @ att_src ; a_dst = g @ att_dst              # [N]
    with self-loops appended, per edge (s -> d):
        e = leaky_relu(a_src[s] + a_dst[d], 0.2)
        alpha = segment_softmax(e, by d)
    out[d] = sum_e alpha_e * g[s_e] + b_gat                # [N, 64]

Sharding: nodes split contiguously across 8 cores (6250 each); edges
partitioned by destination core. Per-core nodes are degree-sorted so
128-node tiles have near-uniform degree.

Key idea vs the 2-half baseline: a SINGLE table (50176 rows of 256 B:
[g bf16 x64 | u=e^{a_src} | v=e^{0.2 a_src} | pad]) gathered with
per-call base offsets. Within each dst its slots are source-sorted, so
the k-th slot across a tile's 128 dsts concentrates in a gid quantile
band; each 8-strip gather call gets its own 32768-row window (int16
index range), eliminating the source-half split (-29% gathered bytes).
The few out-of-window "straggler" edges go to per-tile overflow strips.
Edge weight on-chip: w = max(u*p_d, v*q_d) with p=e^{a_dst}, q=e^{0.2
a_dst} per-partition scalars (exp moved to the node phase; identity
e^{lrelu(x)} = max(e^x, e^{0.2x}), softmax computed without max-shift).
Pad slots point at a pad row with u=v=0 so their weight is exactly 0.
"""

import numpy as np
import ml_dtypes

bf16 = ml_dtypes.bfloat16

R = 8            # cores
P = 128          # partitions / tile size
W_ROW = 128      # table row width in bf16 elems (256 B)
ASRC_F32 = 32    # f32 column of a_src within a row (byte offset 128)
WIN = 32768      # int16 index window (rows per gather call)
STRIP = 8        # k-slots per gather call (8*128 = 1024 descriptors)


class Cfg:
    def __init__(self, N, E, IN=512, EMB=128, OUT=64):
        assert N % R == 0
        self.N, self.E, self.IN, self.EMB, self.OUT = N, E, IN, EMB, OUT
        self.NL = N // R
        self.NL_pad = ((self.NL + 2 + P - 1) // P) * P
        self.TILES = self.NL_pad // P
        self.NTAB = self.NL_pad * R
        self.NCH = 7                         # all-gather chunks
        assert self.TILES % self.NCH == 0
        self.TPC = self.TILES // self.NCH    # tiles per chunk
        self.CH = self.NL_pad // self.NCH    # rows per chunk per core
        npad = self.NL_pad - self.NL
        base_pads = npad // self.NCH
        self.pads = [base_pads + (1 if c < npad % self.NCH else 0)
                     for c in range(self.NCH)]
        pos = np.arange(self.NL_pad)
        inchunk = pos % self.CH
        self.is_pad = np.zeros(self.NL_pad, bool)
        for c in range(self.NCH):
            self.is_pad[(pos // self.CH == c)
                        & (inchunk >= self.CH - self.pads[c])] = True
        self.positions_real = pos[~self.is_pad]
        assert len(self.positions_real) == self.NL


CFG_REAL = Cfg(N=50000, E=1600000)


# --------------------------------------------------------------------------
# host-side preprocessing
# --------------------------------------------------------------------------

def _wrap_idx(lin):
    """dma_gather index layout: linear i -> [i % 16, i // 16], replicated
    across the 8 Q7 core groups -> [128, len/16] int16."""
    assert len(lin) % 16 == 0
    w = lin.reshape(-1, 16).T.astype(np.int16)
    return np.tile(w, (8, 1))


def _pad_row_in(base, cfg):
    """A table row with a_src sentinel inside [base, base+WIN): the last
    row of every (chunk, core) subblock is a pad row."""
    s = base + ((cfg.CH - 1 - base) % cfg.CH)
    assert base <= s < base + WIN and s < cfg.NTAB
    return s


def _build_layout(cfg, src, dst, deg):
    """Slot grids, call windows and per-core index blocks (graph metadata
    is core-uniform; index contents are per-core)."""
    N, NL, NL_pad, TILES = cfg.N, cfg.NL, cfg.NL_pad, cfg.TILES

    CH, NCH = cfg.CH, cfg.NCH
    pos_of = np.empty(N, dtype=np.int64)
    orders = []
    for r in range(R):
        dloc = deg[r * NL:(r + 1) * NL]
        order = np.argsort(-dloc, kind="stable")
        orders.append(order)
        pos_of[r * NL + order] = cfg.positions_real
    core_of = np.arange(N) // NL
    # chunk-major table: gid = chunk*R*CH + core*CH + pos%CH
    gid_of = (pos_of // CH) * R * CH + core_of * CH + (pos_of % CH)

    grids = []
    for r in range(R):
        m = (dst >= r * NL) & (dst < (r + 1) * NL)
        s_r = gid_of[src[m]]
        dpos = pos_of[dst[m]]
        order = np.lexsort((s_r, dpos))
        s_s, d_s = s_r[order], dpos[order]
        starts = np.zeros(NL_pad + 1, np.int64)
        np.add.at(starts, d_s + 1, 1)
        starts = np.cumsum(starts)
        k_of = np.arange(len(s_s)) - starts[d_s]
        grids.append((s_s, d_s, k_of))

    degs_sorted = [np.sort(deg[r * NL:(r + 1) * NL])[::-1] for r in range(R)]
    tile_of_rank = cfg.positions_real // P        # rank i -> tile
    D = np.ones(TILES, np.int64)
    for t in range(TILES):
        rk = np.nonzero(tile_of_rank == t)[0]
        if rk.size:
            D[t] = max(max(int(ds[rk].max()) for ds in degs_sorted), 1)

    tile_meta = []      # per tile: (D_eff, [(k0, nk, base), ...])
    idx_blocks = [[] for _ in range(R)]   # per core: list of wrapped blocks

    for t in range(TILES):
        Dt = int(D[t])
        G = np.full((R, P, Dt), -1, np.int64)
        for r in range(R):
            s_s, d_s, k_of = grids[r]
            mm = (d_s >= t * P) & (d_s < (t + 1) * P)
            G[r, d_s[mm] - t * P, k_of[mm]] = s_s[mm]

        calls = []
        stragglers = []   # (r, d, gid)

        def emit(k0, nk):
            blk = G[:, :, k0:k0 + nk]
            v = blk[blk >= 0]
            if v.size == 0:
                calls.append((k0, nk, 0))
                return
            vs = np.sort(v)
            if vs[-1] - vs[0] < WIN:
                calls.append((k0, nk, int(vs[0])))
                return
            if nk > 1:
                h = nk // 2
                emit(k0, h)
                emit(k0 + h, nk - h)
                return
            # single strip still over-span: max-coverage window
            hi_idx = np.searchsorted(vs, vs + WIN, side="left")
            cover = hi_idx - np.arange(len(vs))
            i0 = int(np.argmax(cover))
            base = int(vs[i0])
            bad = (blk >= 0) & ((blk < base) | (blk >= base + WIN))
            for r, dd, kk in zip(*np.nonzero(bad)):
                stragglers.append((int(r), int(dd), int(blk[r, dd, kk])))
                G[r, dd, k0 + kk] = -1
            calls.append((k0, nk, base))

        for k0 in range(0, Dt, STRIP):
            emit(k0, min(STRIP, Dt - k0))

        # re-home stragglers: free (sentinel) slot in a window-matching
        # call, else swap with a same-dst edge that fits both windows
        leftover = []
        for r, dd, gid in stragglers:
            placed = False
            for (k0, nk, base) in calls:
                if base <= gid < base + WIN:
                    free = np.nonzero(G[r, dd, k0:k0 + nk] < 0)[0]
                    if free.size:
                        G[r, dd, k0 + int(free[0])] = gid
                        placed = True
                        break
            if not placed:
                for (k0, nk, base) in calls:
                    if placed or not (base <= gid < base + WIN):
                        continue
                    for kk in range(nk):
                        other = int(G[r, dd, k0 + kk])
                        if other < 0:
                            continue
                        # move `other` elsewhere so `gid` can take its slot
                        for (k2, nk2, base2) in calls:
                            if (k2, nk2, base2) == (k0, nk, base):
                                continue
                            if base2 <= other < base2 + WIN:
                                free2 = np.nonzero(
                                    G[r, dd, k2:k2 + nk2] < 0)[0]
                                if free2.size:
                                    G[r, dd, k2 + int(free2[0])] = other
                                    G[r, dd, k0 + kk] = gid
                                    placed = True
                                    break
                        if placed:
                            break
            if not placed:
                leftover.append((r, dd, gid))
        # overflow strips (shared windows across cores)
        ov = []   # list of dicts: {base, slots: {(r, d): gid}}
        leftover.sort(key=lambda x: x[2])
        for r, dd, gid in leftover:
            for s in ov:
                if s["base"] <= gid < s["base"] + WIN and (r, dd) not in s["slots"]:
                    s["slots"][(r, dd)] = gid
                    break
            else:
                ov.append({"base": gid, "slots": {(r, dd): gid}})
        Dov = len(ov)
        De = Dt + Dov
        Gf = np.full((R, P, De), -1, np.int64)
        Gf[:, :, :Dt] = G
        for j, s in enumerate(ov):
            for (r, dd), gid in s["slots"].items():
                Gf[r, dd, Dt + j] = gid
            calls.append((Dt + j, 1, int(s["base"])))

        tile_meta.append((De, calls))

        for r in range(R):
            for (k0, nk, base) in calls:
                pad = _pad_row_in(base, cfg)
                lin = Gf[r, :, k0:k0 + nk].T.reshape(-1).copy()  # i = kk*128+p
                lin[lin < 0] = pad
                lin -= base
                assert lin.min() >= 0 and lin.max() < WIN
                idx_blocks[r].append(_wrap_idx(lin))

    offs = [np.ascontiguousarray(np.concatenate(b, axis=1)) for b in idx_blocks]
    return orders, gid_of, tile_meta, offs


def _prepare(cfg, x, edge_index, W_dense, b_dense, W_gat, att_src, att_dst,
             b_gat):
    N, NL, NL_pad, TILES = cfg.N, cfg.NL, cfg.NL_pad, cfg.TILES
    src = edge_index[0].astype(np.int64)
    dst = edge_index[1].astype(np.int64)
    loops = np.arange(N, dtype=np.int64)
    src = np.concatenate([src, loops])
    dst = np.concatenate([dst, loops])
    deg = np.bincount(dst, minlength=N)

    orders, gid_of, tile_meta, offs = _build_layout(cfg, src, dst, deg)

    wdT = np.ascontiguousarray(W_dense.T)            # [IN, EMB]
    wdT_packed = np.concatenate(
        [wdT[k * P:(k + 1) * P, :] for k in range(cfg.IN // P)], axis=1)
    att = np.stack([att_src, att_dst], axis=1)       # [OUT, 2]
    KC = cfg.IN // P

    in_maps = []
    for r in range(R):
        xp = np.zeros((NL_pad, cfg.IN), dtype=np.float32)
        xp[cfg.positions_real] = x[r * NL + orders[r]]
        xT = np.empty((P, TILES * KC * P), dtype=bf16)
        for t in range(TILES):
            blk = xp[t * P:(t + 1) * P, :].T.astype(bf16)   # [IN, P]
            xT[:, (t * KC) * P:(t + 1) * KC * P] = \
                blk.reshape(KC, P, P).transpose(1, 0, 2).reshape(P, KC * P)
        in_maps.append({
            "xT": xT,
            "wdT": wdT_packed.astype(bf16),
            "bd": b_dense.reshape(cfg.EMB, 1).astype(np.float32),
            "wgT": np.ascontiguousarray(W_gat.T).astype(bf16),
            "att": att.astype(bf16),
            "bgat": b_gat.reshape(cfg.OUT, 1).astype(np.float32),
            "offs": offs[r],
        })
    return in_maps, orders, tile_meta


def _assemble(cfg, results, orders):
    out = np.empty((cfg.N, cfg.OUT), dtype=np.float32)
    for r in range(R):
        o = results[r]["out"][cfg.positions_real]
        out[r * cfg.NL + orders[r]] = o
    return out


# --------------------------------------------------------------------------
# device graph
# --------------------------------------------------------------------------

def _build_graph(cfg, tile_meta):
    import concourse.bass as bass
    import concourse.bacc as bacc
    import concourse.mybir as mybir
    import concourse.tile as tile
    from concourse.masks import make_identity

    IN, EMB, OUT = cfg.IN, cfg.EMB, cfg.OUT
    KC = IN // P
    TILES, NL_pad, NTAB, NL = cfg.TILES, cfg.NL_pad, cfg.NTAB, cfg.NL
    TOTD = sum(m[0] for m in tile_meta)
    fp32 = mybir.dt.float32
    b16 = mybir.dt.bfloat16
    i16 = mybir.dt.int16

    nc = bacc.Bacc(None, target_bir_lowering=False, debug=False, num_devices=R,
                   num_swdge_queues=4)

    xT = nc.dram_tensor("xT", [P, TILES * KC * P], b16, kind="ExternalInput")
    wdT = nc.dram_tensor("wdT", [P, KC * EMB], b16, kind="ExternalInput")
    bd = nc.dram_tensor("bd", [EMB, 1], fp32, kind="ExternalInput")
    wgT = nc.dram_tensor("wgT", [EMB, OUT], b16, kind="ExternalInput")
    att = nc.dram_tensor("att", [OUT, 2], b16, kind="ExternalInput")
    bgat = nc.dram_tensor("bgat", [OUT, 1], fp32, kind="ExternalInput")
    offs_ext = nc.dram_tensor("offs", [P, 8 * TOTD], i16, kind="ExternalInput")
    out = nc.dram_tensor("out", [NL_pad, OUT], fp32, kind="ExternalOutput")

    with tile.TileContext(nc) as tc:
        with (
            tc.tile_pool(name="dram", bufs=1, space="DRAM") as dram,
            tc.tile_pool(name="const", bufs=1) as cst,
        ):
            shard = dram.tile([NL_pad, W_ROW], b16)
            full = dram.tile([NTAB, W_ROW], b16)

            identb = cst.tile([P, P], b16)
            make_identity(nc, identb[:])
            identf = cst.tile([P, P], fp32)
            make_identity(nc, identf[:])

            wdTs = cst.tile([P, KC * EMB], b16)
            nc.sync.dma_start(out=wdTs[:], in_=wdT[:, :])
            bds = cst.tile([EMB, 1], fp32)
            nc.sync.dma_start(out=bds[:], in_=bd[:, :])
            wgTs = cst.tile([EMB, OUT], b16)
            nc.sync.dma_start(out=wgTs[:], in_=wgT[:, :])
            atts = cst.tile([OUT, 2], b16)
            nc.sync.dma_start(out=atts[:], in_=att[:, :])
            attmat = cst.tile([P, 2 * OUT], b16)
            bgs = cst.tile([OUT, 1], fp32)
            nc.sync.dma_start(out=bgs[:], in_=bgat[:, :])
            adst_all = cst.tile([P, TILES], fp32)
            zuv = cst.tile([max(cfg.pads), 1], fp32)
            nc.vector.memset(zuv[:], -80.0)
            adst02_all = cst.tile([P, TILES], fp32)

            # ---------------- node phase ----------------
            with (
                tc.tile_pool(name="npsum_h", bufs=2, space="PSUM") as ps_h,
                tc.tile_pool(name="npsum_m", bufs=1, space="PSUM") as ps_m,
                tc.tile_pool(name="nsb", bufs=3) as nsb,
            ):
                bgp = ps_m.tile([P, OUT], fp32, tag="misc")
                nc.tensor.transpose(out=bgp[:], in_=bgs[:].to_broadcast([OUT, P]),
                                    identity=identf[:OUT, :OUT])
                bgmat = cst.tile([P, OUT], fp32)
                nc.vector.tensor_copy(bgmat[:], bgp[:])
                # att rows replicated across partitions: [att_src | att_dst]
                amp = ps_m.tile([P, 2 * OUT], b16, tag="misc")
                nc.tensor.transpose(
                    out=amp[:, 0:OUT], in_=atts[:, 0:1].to_broadcast([OUT, P]),
                    identity=identb[:OUT, :OUT])
                nc.tensor.transpose(
                    out=amp[:, OUT:2 * OUT],
                    in_=atts[:, 1:2].to_broadcast([OUT, P]),
                    identity=identb[:OUT, :OUT])
                nc.vector.tensor_copy(attmat[:], amp[:])

                for t in range(TILES):
                    xTs = nsb.tile([P, KC * P], b16, tag="xTs")
                    nc.sync.dma_start(
                        out=xTs[:], in_=xT[:, t * KC * P:(t + 1) * KC * P])
                    hTp = ps_h.tile([EMB, P], fp32, tag="hT")
                    for k in range(KC):
                        nc.tensor.matmul(out=hTp[:],
                                         lhsT=wdTs[:, k * EMB:(k + 1) * EMB],
                                         rhs=xTs[:, k * P:(k + 1) * P],
                                         start=(k == 0), stop=(k == KC - 1))
                    u = nsb.tile([EMB, P], fp32, tag="u")
                    nc.scalar.activation(u[:], hTp[:],
                                         mybir.ActivationFunctionType.Identity,
                                         bias=bds[:, :1])
                    hT = nsb.tile([EMB, P], b16, tag="hT_sb")
                    nc.vector.scalar_tensor_tensor(
                        out=hT[:], in0=u[:], scalar=0.01, in1=u[:],
                        op0=mybir.AluOpType.mult, op1=mybir.AluOpType.max)
                    gTp = ps_m.tile([OUT, P], fp32, tag="misc")
                    nc.tensor.matmul(out=gTp[:], lhsT=wgTs[:], rhs=hT[:],
                                     start=True, stop=True)
                    stg = nsb.tile([OUT, P], b16, tag="stg")
                    nc.vector.tensor_copy(stg[:], gTp[:])
                    # transpose gT -> table g block
                    ttp = ps_m.tile([P, OUT], b16, tag="ttp")
                    nc.tensor.transpose(out=ttp[:], in_=stg[:],
                                        identity=identb[:OUT, :OUT])
                    tabs = nsb.tile([P, W_ROW], b16, tag="tabs")
                    nc.scalar.activation(tabs[:, 0:OUT], ttp[:],
                                         mybir.ActivationFunctionType.Copy)
                    # a_src/a_dst as per-partition row-dots with att columns
                    gw = nsb.tile([P, 2 * OUT], fp32, tag="gw")
                    nc.vector.tensor_tensor(
                        out=gw[:, 0:OUT], in0=tabs[:, 0:OUT],
                        in1=attmat[:, 0:OUT], op=mybir.AluOpType.mult)
                    nc.vector.tensor_tensor(
                        out=gw[:, OUT:2 * OUT], in0=tabs[:, 0:OUT],
                        in1=attmat[:, OUT:2 * OUT], op=mybir.AluOpType.mult)
                    nc.vector.tensor_reduce(
                        out=tabs[:].bitcast(fp32)[:, ASRC_F32:ASRC_F32 + 1],
                        in_=gw[:, 0:OUT], op=mybir.AluOpType.add,
                        axis=mybir.AxisListType.X)
                    nc.vector.tensor_reduce(
                        out=adst_all[:, t:t + 1],
                        in_=gw[:, OUT:2 * OUT], op=mybir.AluOpType.add,
                        axis=mybir.AxisListType.X)
                    nc.sync.dma_start(
                        out=shard[t * P:(t + 1) * P, :], in_=tabs[:])

                    # chunk boundary: pad sentinels; all-gather in 2 groups
                    if (t + 1) % cfg.TPC == 0:
                        c = t // cfg.TPC
                        pc = cfg.pads[c]
                        lo = (c + 1) * cfg.CH - pc
                        nc.sync.dma_start(
                            out=shard[lo:lo + pc, :].bitcast(fp32)
                                [:, ASRC_F32:ASRC_F32 + 1],
                            in_=zuv[:pc, :])
                        nc.gpsimd.collective_compute(
                            "AllGather", mybir.AluOpType.bypass,
                            replica_groups=[list(range(R))],
                            ins=[shard[c * cfg.CH:(c + 1) * cfg.CH, :].opt()],
                            outs=[full[c * R * cfg.CH:
                                       (c + 1) * R * cfg.CH, :].opt()],
                        )

                nc.vector.tensor_scalar(
                    out=adst02_all[:], in0=adst_all[:], scalar1=0.2,
                    scalar2=None, op0=mybir.AluOpType.mult)

            # ---------------- edge phase ----------------
            qi = 0
            allidx = cst.tile([P, 8 * TOTD], i16)
            nc.sync.dma_start(out=allidx[:], in_=offs_ext[:, :])
            with tc.tile_pool(name="esb", bufs=3) as esb:
                cum = 0
                for t in range(TILES):
                    De, calls = tile_meta[t]
                    Dq = De
                    rows = esb.tile([P, Dq * W_ROW], b16, tag="rows")
                    for (k0, nk, base) in calls:
                        hi = min(base + WIN, NTAB)
                        nc.gpsimd.dma_gather(
                            out_ap=rows[:, k0 * W_ROW:(k0 + nk) * W_ROW]
                                .rearrange("p (j e) -> p j e", e=W_ROW),
                            in_ap=full[base:hi, :],
                            idxs_ap=allidx[:, 8 * (cum + k0):
                                           8 * (cum + k0 + nk)],
                            num_idxs=nk * P, num_idxs_reg=nk * P,
                            elem_size=W_ROW,
                            single_packet=False,
                            queue_num=qi % 4,
                        )
                        qi += 1
                    rowsv = rows[:].rearrange("p (d e) -> p d e", e=W_ROW)
                    asrc = rows[:].bitcast(fp32).rearrange(
                        "p (d e) -> p d e", e=W_ROW // 2)[:, :, ASRC_F32]
                    # w = e^{lrelu(a_src+a_dst, 0.2)} = max(e^s, e^{0.2 s})
                    t1 = esb.tile([P, Dq], fp32, tag="t1")
                    nc.scalar.activation(t1[:], asrc,
                                         mybir.ActivationFunctionType.Exp,
                                         bias=adst_all[:, t:t + 1], scale=1.0)
                    t2 = esb.tile([P, Dq], fp32, tag="t2")
                    nc.scalar.activation(t2[:], asrc,
                                         mybir.ActivationFunctionType.Exp,
                                         bias=adst02_all[:, t:t + 1], scale=0.2)
                    w = esb.tile([P, Dq], fp32, tag="w")
                    nc.vector.tensor_tensor(out=w[:], in0=t1[:], in1=t2[:],
                                            op=mybir.AluOpType.max)
                    denom = esb.tile([P, 1], fp32, tag="denom")
                    nc.vector.tensor_reduce(out=denom[:], in_=w[:],
                                            op=mybir.AluOpType.add,
                                            axis=mybir.AxisListType.X)
                    rden = esb.tile([P, 1], fp32, tag="rden")
                    nc.vector.reciprocal(rden[:], denom[:])
                    gsc = esb.tile([P, Dq * OUT], b16, tag="gsc")
                    nc.vector.tensor_tensor(
                        out=gsc[:].rearrange("p (d c) -> p d c", c=OUT),
                        in0=rowsv[:, :, 0:OUT],
                        in1=w[:].to_broadcast([P, Dq, OUT]),
                        op=mybir.AluOpType.mult)
                    onum = esb.tile([P, OUT], fp32, tag="onum")
                    nc.vector.tensor_reduce(
                        out=onum[:],
                        in_=gsc[:].rearrange("p (d c) -> p c d", c=OUT),
                        op=mybir.AluOpType.add, axis=mybir.AxisListType.X)
                    outf = esb.tile([P, OUT], fp32, tag="outf")
                    nc.vector.scalar_tensor_tensor(
                        out=outf[:], in0=onum[:], scalar=rden[:, :1],
                        in1=bgmat[:],
                        op0=mybir.AluOpType.mult, op1=mybir.AluOpType.add)
                    nc.sync.dma_start(out=out[t * P:(t + 1) * P, :], in_=outf[:])
                    cum += De
    nc.finalize()
    return nc


# --------------------------------------------------------------------------
# entry points
# --------------------------------------------------------------------------

def run(inputs, cfg=CFG_REAL, trace=False):
    from concourse.bass_utils import run_bass_kernel_spmd
    in_maps, orders, tile_meta = _prepare(cfg, **inputs)
    nc = _build_graph(cfg, tile_meta)
    res = run_bass_kernel_spmd(nc, in_maps, core_ids=list(range(R)),
                               trace=trace)
    out = _assemble(cfg, res.results, orders)
    return out, res


def kernel(**inputs):
    inputs = {k: np.asarray(v) for k, v in inputs.items()}
    out, _ = run(inputs, CFG_REAL, trace=False)
    return out

